# revision 6
# baseline (speedup 1.0000x reference)
"""Trainium2 Bass kernel for nn_HadamardModule (SORF random-feature module).

Reference computation:
    x_ = x @ projector                      # [N=8192, 128]
    y = broadcast over 64 stacks
    for t in 0,1: y = COEFF * fwht(d[t] * y)
    out = cos(y.reshape(N, 8192) + 2*pi*b)

Key identity: fwht over 128 elems == multiply by symmetric Hadamard matrix H.
The whole per-stack SORF transform is linear:
    feats[:, s] = x_ @ A_s,   A_s = COEFF^2 * diag(d0_s) @ H @ diag(d1_s) @ H
A_s is folded on the host (tiny); the device does:
    z0 = (x @ P) @ (A_s / 2pi)            # phase in periods, via TensorE fp32
    r  = z0' - round(z0')                 # range reduction, z0' = z0 + frac-bias
    out = sin(2*pi*r)                     # ScalarE Sin LUT (valid on [-pi, pi])
round() uses the fp32 magic-number trick ((v + 1.5*2^23) - 1.5*2^23), in
2 VectorE passes; the per-feature bias rides in the first pass and in the
Sin activation's per-partition bias so everything stays exact:
    tmid = (z0 + b'') + M                  # = round(z0 + b'') + M, exact
    u    = (tmid - M) - z0                 # = b'' - r, exact (cancellation)
    out  = Sin(-2pi * u + 2pi * b'')       # = sin(2pi * r),  arg in [-pi, pi]

Sharding: data-parallel over the 8192 rows -> 1024 rows per core on 8 cores.
x is passed pre-transposed (features on partitions) so no device transposes
are needed; the output comes back feature-major per 128-feature stack block
and is re-assembled on the host.
"""

import numpy as np

NPCAS = 128
OUT_DIM = 8192
NSTACKS = 64
COEFF = np.sqrt(np.float64(NPCAS)) / 3.0
TWO_PI = 2.0 * np.pi
N_CORES = 8
ROWS = 8192
ROWS_PER_CORE = ROWS // N_CORES  # 1024
CHUNK = 512
N_CHUNKS = ROWS_PER_CORE // CHUNK  # 2
MAGIC = float(np.float32(1.5 * 2**23))

_cached = {}


def _hadamard128():
    H = np.array([[1.0]])
    while H.shape[0] < NPCAS:
        H = np.block([[H, H], [H, -H]])
    return H


def _build_nc():
    import concourse.bacc as bacc
    import concourse.mybir as mybir
    import concourse.tile as tile

    f32 = mybir.dt.float32
    add = mybir.AluOpType.add
    sub = mybir.AluOpType.subtract

    nc = bacc.Bacc("TRN2", target_bir_lowering=False, debug=False)
    xT = nc.dram_tensor("xT", [4, 128, ROWS_PER_CORE], f32, kind="ExternalInput")
    Pc = nc.dram_tensor("Pc", [4, 128, 128], f32, kind="ExternalInput")
    Ad = nc.dram_tensor("Ad", [NSTACKS, 128, 128], f32, kind="ExternalInput")
    b1d = nc.dram_tensor("b1d", [128, NSTACKS], f32, kind="ExternalInput")
    b2d = nc.dram_tensor("b2d", [128, NSTACKS], f32, kind="ExternalInput")
    out = nc.dram_tensor(
        "out", [NSTACKS, N_CHUNKS, 128, CHUNK], f32, kind="ExternalOutput"
    )

    with tile.TileContext(nc) as tc:
        with (
            tc.tile_pool(name="const", bufs=1) as const,
            tc.tile_pool(name="psum_p", bufs=2, space="PSUM") as psum_p,
            tc.tile_pool(name="psum_z", bufs=4, space="PSUM") as psum_z,
            tc.tile_pool(name="work", bufs=4) as work,
            tc.tile_pool(name="outp", bufs=6) as outp,
        ):
            Pt = const.tile([128, 4, 128], f32)
            Xt = const.tile([128, 4, ROWS_PER_CORE], f32)
            for k in range(4):
                nc.sync.dma_start(Pt[:, k, :], Pc[k])
                nc.sync.dma_start(Xt[:, k, :], xT[k])
            At = const.tile([128, NSTACKS, 128], f32)
            for s in range(NSTACKS):
                nc.sync.dma_start(At[:, s, :], Ad[s])
            b1 = const.tile([128, NSTACKS], f32)
            b2 = const.tile([128, NSTACKS], f32)
            nc.sync.dma_start(b1[:], b1d[:])
            nc.sync.dma_start(b2[:], b2d[:])

            # projection: x_^T = P^T @ x^T, K=512 in 4 chunks of 128
            xsb = const.tile([128, N_CHUNKS, CHUNK], f32)
            for c in range(N_CHUNKS):
                pp = psum_p.tile([128, CHUNK], f32)
                for k in range(4):
                    nc.tensor.matmul(
                        pp[:],
                        Pt[:, k, :],
                        Xt[:, k, c * CHUNK : (c + 1) * CHUNK],
                        start=(k == 0),
                        stop=(k == 3),
                    )
                nc.vector.tensor_copy(xsb[:, c, :], pp[:])

            # per-stack: z0 = A_s^T/2pi applied to x_^T; range-reduce; Sin
            for s in range(NSTACKS):
                for c in range(N_CHUNKS):
                    z0 = psum_z.tile([128, CHUNK], f32)
                    nc.tensor.matmul(
                        z0[:], At[:, s, :], xsb[:, c, :], start=True, stop=True
                    )
                    tmid = work.tile([128, CHUNK], f32, tag="tmid")
                    nc.vector.tensor_scalar(
                        tmid[:], z0[:], b1[:, s : s + 1], MAGIC, add, add
                    )
                    u = work.tile([128, CHUNK], f32, tag="u")
                    nc.vector.scalar_tensor_tensor(
                        u[:], tmid[:], MAGIC, z0[:], sub, sub
                    )
                    osb = outp.tile([128, CHUNK], f32)
                    nc.scalar.activation(
                        osb[:],
                        u[:],
                        mybir.ActivationFunctionType.Sin,
                        bias=b2[:, s : s + 1],
                        scale=-TWO_PI,
                    )
                    nc.sync.dma_start(out[s, c], osb[:])

    nc.compile()
    return nc


def _make_runner():
    """Compile once and build a persistent jitted SPMD executable.

    Adapted from concourse.bass2jax.run_bass_via_pjrt, but cached across
    calls: x shards across the 8 cores, the small operands broadcast, and
    the donated zero output buffers are created on-device.
    """
    import jax
    import jax.numpy as jnp
    import concourse.mybir as mybir
    from jax.experimental.shard_map import shard_map
    from jax.sharding import Mesh, PartitionSpec
    from concourse.bass2jax import (
        _bass_exec_p,
        install_neuronx_cc_hook,
        partition_id_tensor,
    )

    nc = _build_nc()
    install_neuronx_cc_hook()

    partition_name = (
        nc.partition_id_tensor.name if nc.partition_id_tensor else None
    )
    in_names, out_names, out_avals = [], [], []
    for alloc in nc.m.functions[0].allocations:
        if not isinstance(alloc, mybir.MemoryLocationSet):
            continue
        name = alloc.memorylocations[0].name
        if alloc.kind == "ExternalInput":
            if name != partition_name:
                in_names.append(name)
        elif alloc.kind == "ExternalOutput":
            out_names.append(name)
            out_avals.append(
                jax.core.ShapedArray(
                    tuple(alloc.tensor_shape), mybir.dt.np(alloc.dtype)
                )
            )

    sharded_inputs = {"xT"}
    call_names = tuple(in_names) + tuple(out_names)
    if partition_name is not None:
        call_names = call_names + (partition_name,)

    def _body(*args):
        extra = [partition_id_tensor()] if partition_name is not None else []
        outs = _bass_exec_p.bind(
            *args,
            *extra,
            out_avals=tuple(out_avals),
            in_names=call_names,
            out_names=tuple(out_names),
            lowering_input_output_aliases=(),
            sim_require_finite=True,
            sim_require_nnan=True,
            nc=nc,
        )
        return tuple(outs)

    devices = jax.devices()[:N_CORES]
    mesh = Mesh(np.asarray(devices), ("core",))
    in_specs = tuple(
        PartitionSpec("core") if n in sharded_inputs else PartitionSpec()
        for n in in_names
    ) + (PartitionSpec("core"),) * len(out_names)
    out_specs = (PartitionSpec("core"),) * len(out_names)
    fn = jax.jit(
        shard_map(
            _body, mesh=mesh, in_specs=in_specs, out_specs=out_specs, check_rep=False
        )
    )

    # device-resident zero output buffers, transferred once and reused
    # (the NEFF writes every element of `out`, so stale contents are fine)
    from jax.sharding import NamedSharding

    zeros = [
        jax.device_put(
            np.zeros((N_CORES * a.shape[0], *a.shape[1:]), a.dtype),
            NamedSharding(mesh, PartitionSpec("core")),
        )
        for a in out_avals
    ]
    return fn, in_names, zeros


def _get_runner():
    if "runner" not in _cached:
        _cached["runner"] = _make_runner()
    return _cached["runner"]


def _host_prep(x, projector, d, b):
    """Fold the SORF transform into per-stack matrices + per-feature biases."""
    H = _hadamard128()
    d64 = d.astype(np.float64)
    # A_s = (COEFF^2/2pi) * diag(d0_s) @ H @ diag(d1_s) @ H  -> [64, 128, 128]
    inner = np.matmul(H[None, :, :] * d64[1][:, None, :], H)
    A = (COEFF**2 / TWO_PI) * d64[0][:, :, None] * inner
    A = np.ascontiguousarray(A, dtype=np.float32)

    # phase bias in periods: b' = b + 0.25 (cos -> sin); b'' = b' - round(b')
    bp = b.astype(np.float64) + 0.25
    bpp = bp - np.round(bp)  # in [-0.5, 0.5]
    b1 = np.ascontiguousarray(bpp.reshape(NSTACKS, 128).T.astype(np.float32))
    b2 = np.ascontiguousarray(
        (TWO_PI * bpp).reshape(NSTACKS, 128).T.astype(np.float32)
    )

    Pc = np.ascontiguousarray(projector.astype(np.float32).reshape(4, 128, 128))

    # global xT: [8*4, 128, 1024]; shard_map slices axis 0 per core
    x2 = x.astype(np.float32).reshape(ROWS, 512)
    xT = np.empty((N_CORES, 4, 128, ROWS_PER_CORE), np.float32)
    for core in range(N_CORES):
        xs = x2[core * ROWS_PER_CORE : (core + 1) * ROWS_PER_CORE]
        xT[core] = xs.T.reshape(4, 128, ROWS_PER_CORE)
    xT = xT.reshape(N_CORES * 4, 128, ROWS_PER_CORE)
    return {"xT": xT, "Pc": Pc, "Ad": A, "b1d": b1, "b2d": b2}


def _assemble(out_global):
    """[8*64, 2, 128, 512] core-sharded -> [64, 128, 8192] full output."""
    o = np.asarray(out_global).reshape(N_CORES, NSTACKS, N_CHUNKS, 128, CHUNK)
    full = np.empty((ROWS, OUT_DIM), np.float32)
    view = full.reshape(N_CORES, N_CHUNKS, CHUNK, NSTACKS, 128)
    # o[core, s, c, m, j] -> view[core, c, j, s, m]
    np.copyto(view, o.transpose(0, 2, 4, 1, 3))
    return full.reshape(64, 128, OUT_DIM)


def kernel(x, projector, d, b):
    fn, in_names, zeros = _get_runner()
    ins = _host_prep(
        np.asarray(x), np.asarray(projector), np.asarray(d), np.asarray(b)
    )
    outs = fn(*[ins[n] for n in in_names], *zeros)
    return _assemble(outs[0])


# revision 7
# speedup vs baseline: 1.8380x; 1.8380x over previous
"""Trainium2 Bass kernel for nn_HadamardModule (SORF random-feature module).

Reference computation:
    x_ = x @ projector                      # [N=8192, 128]
    y = broadcast over 64 stacks
    for t in 0,1: y = COEFF * fwht(d[t] * y)
    out = cos(y.reshape(N, 8192) + 2*pi*b)

Key identity: fwht over 128 elems == multiply by symmetric Hadamard matrix H.
The whole per-stack SORF transform is linear:
    feats[:, s] = x_ @ A_s,   A_s = COEFF^2 * diag(d0_s) @ H @ diag(d1_s) @ H
A_s/2pi is folded ON DEVICE (H @ (d1_s * H) is an exact integer matmul, then a
per-partition scale by COEFF^2/(2pi) * d0_s); the main loop computes:
    z0 = (x @ P) @ (A_s / 2pi)            # phase in periods, via TensorE fp32
    r  = z0' - round(z0')                 # range reduction, z0' = z0 + frac-bias
    out = sin(2*pi*r)                     # ScalarE Sin LUT (valid on [-pi, pi])
round() uses the fp32 magic-number trick ((v + 1.5*2^23) - 1.5*2^23), in
2 VectorE passes; the per-feature bias rides in the first pass and in the
Sin activation's per-partition bias so everything stays exact:
    tmid = (z0 + b'') + M                  # = round(z0 + b'') + M, exact
    u    = (tmid - M) - z0                 # = b'' - r, exact (cancellation)
    out  = Sin(-2pi * u + 2pi * b'')       # = sin(2pi * r),  arg in [-pi, pi]
Output is written bf16 (quantization ~1e-3, well under the ~9e-3 fp32 noise
floor of this phase-sensitive computation) and upcast on the host.

Sharding: data-parallel over the 8192 rows -> 1024 rows per core on 8 cores.
x is passed pre-transposed (features on partitions) so no device transposes
are needed; the output comes back feature-major per 128-feature stack block
and is re-assembled on the host.
"""

import concurrent.futures as _futures

import numpy as np

NPCAS = 128
OUT_DIM = 8192
NSTACKS = 64
COEFF = np.sqrt(np.float64(NPCAS)) / 3.0
TWO_PI = 2.0 * np.pi
C_SCALE = float(COEFF**2 / TWO_PI)
N_CORES = 8
ROWS = 8192
ROWS_PER_CORE = ROWS // N_CORES  # 1024
CHUNK = 512
N_CHUNKS = ROWS_PER_CORE // CHUNK  # 2
MAGIC = float(np.float32(1.5 * 2**23))

_cached = {}


def _hadamard128():
    H = np.array([[1.0]])
    while H.shape[0] < NPCAS:
        H = np.block([[H, H], [H, -H]])
    return H


def _build_nc():
    import concourse.bacc as bacc
    import concourse.mybir as mybir
    import concourse.tile as tile

    f32 = mybir.dt.float32
    bf16 = mybir.dt.bfloat16
    add = mybir.AluOpType.add
    sub = mybir.AluOpType.subtract
    mult = mybir.AluOpType.mult

    nc = bacc.Bacc("TRN2", target_bir_lowering=False, debug=False)
    xT = nc.dram_tensor("xT", [4, 128, ROWS_PER_CORE], f32, kind="ExternalInput")
    Pc = nc.dram_tensor("Pc", [4, 128, 128], f32, kind="ExternalInput")
    Hd = nc.dram_tensor("Hd", [128, 128], f32, kind="ExternalInput")
    d0d = nc.dram_tensor("d0d", [128, NSTACKS], f32, kind="ExternalInput")
    d1d = nc.dram_tensor("d1d", [128, NSTACKS], f32, kind="ExternalInput")
    b1d = nc.dram_tensor("b1d", [128, NSTACKS], f32, kind="ExternalInput")
    b2d = nc.dram_tensor("b2d", [128, NSTACKS], f32, kind="ExternalInput")
    out = nc.dram_tensor(
        "out", [NSTACKS, N_CHUNKS, 128, CHUNK], bf16, kind="ExternalOutput"
    )

    with tile.TileContext(nc) as tc:
        with (
            tc.tile_pool(name="const", bufs=1) as const,
            tc.tile_pool(name="psum_f", bufs=2, space="PSUM") as psum_f,
            tc.tile_pool(name="psum_p", bufs=2, space="PSUM") as psum_p,
            tc.tile_pool(name="psum_z", bufs=4, space="PSUM") as psum_z,
            tc.tile_pool(name="fold", bufs=2) as foldp,
            tc.tile_pool(name="work", bufs=4) as work,
            tc.tile_pool(name="outp", bufs=6) as outp,
        ):
            Pt = const.tile([128, 4, 128], f32)
            Xt = const.tile([128, 4, ROWS_PER_CORE], f32)
            for k in range(4):
                nc.sync.dma_start(Pt[:, k, :], Pc[k])
                nc.sync.dma_start(Xt[:, k, :], xT[k])
            Ht = const.tile([128, 128], f32)
            nc.sync.dma_start(Ht[:], Hd[:])
            d0t = const.tile([128, NSTACKS], f32)
            d1t = const.tile([128, NSTACKS], f32)
            nc.sync.dma_start(d0t[:], d0d[:])
            nc.sync.dma_start(d1t[:], d1d[:])
            b1 = const.tile([128, NSTACKS], f32)
            b2 = const.tile([128, NSTACKS], f32)
            nc.sync.dma_start(b1[:], b1d[:])
            nc.sync.dma_start(b2[:], b2d[:])

            # fold A_s/2pi = (C*d0_s) * (H @ (d1_s * H)) on device.
            # H @ (d1*H) is exact (integer entries <= 128 in fp32 accum);
            # d0t is pre-scaled by C_SCALE on the host.
            At = const.tile([128, NSTACKS, 128], f32)
            for s in range(NSTACKS):
                w1 = foldp.tile([128, 128], f32, tag="w1")
                nc.vector.tensor_scalar(w1[:], Ht[:], d1t[:, s : s + 1], None, mult)
                pin = psum_f.tile([128, 128], f32)
                nc.tensor.matmul(pin[:], Ht[:], w1[:], start=True, stop=True)
                nc.vector.tensor_scalar(
                    At[:, s, :], pin[:], d0t[:, s : s + 1], None, mult
                )

            # projection: x_^T = P^T @ x^T, K=512 in 4 chunks of 128
            xsb = const.tile([128, N_CHUNKS, CHUNK], f32)
            for c in range(N_CHUNKS):
                pp = psum_p.tile([128, CHUNK], f32)
                for k in range(4):
                    nc.tensor.matmul(
                        pp[:],
                        Pt[:, k, :],
                        Xt[:, k, c * CHUNK : (c + 1) * CHUNK],
                        start=(k == 0),
                        stop=(k == 3),
                    )
                nc.vector.tensor_copy(xsb[:, c, :], pp[:])

            # per-stack: z0 = A_s^T/2pi applied to x_^T; range-reduce; Sin
            for s in range(NSTACKS):
                for c in range(N_CHUNKS):
                    z0 = psum_z.tile([128, CHUNK], f32)
                    nc.tensor.matmul(
                        z0[:], At[:, s, :], xsb[:, c, :], start=True, stop=True
                    )
                    tmid = work.tile([128, CHUNK], f32, tag="tmid")
                    nc.vector.tensor_scalar(
                        tmid[:], z0[:], b1[:, s : s + 1], MAGIC, add, add
                    )
                    u = work.tile([128, CHUNK], f32, tag="u")
                    nc.vector.scalar_tensor_tensor(
                        u[:], tmid[:], MAGIC, z0[:], sub, sub
                    )
                    osb = outp.tile([128, CHUNK], bf16)
                    nc.scalar.activation(
                        osb[:],
                        u[:],
                        mybir.ActivationFunctionType.Sin,
                        bias=b2[:, s : s + 1],
                        scale=-TWO_PI,
                    )
                    nc.sync.dma_start(out[s, c], osb[:])

    nc.compile()
    return nc


def _make_runner():
    """Compile once and build a persistent jitted SPMD executable.

    Adapted from concourse.bass2jax.run_bass_via_pjrt, but cached across
    calls: x shards across the 8 cores, the small operands broadcast, and
    the zero output buffers live on device (not donated, reused each call;
    the NEFF overwrites every element of `out`).
    """
    import jax
    import concourse.mybir as mybir
    from jax.experimental.shard_map import shard_map
    from jax.sharding import Mesh, NamedSharding, PartitionSpec
    from concourse.bass2jax import (
        _bass_exec_p,
        install_neuronx_cc_hook,
        partition_id_tensor,
    )

    nc = _build_nc()
    install_neuronx_cc_hook()

    partition_name = (
        nc.partition_id_tensor.name if nc.partition_id_tensor else None
    )
    in_names, out_names, out_avals = [], [], []
    for alloc in nc.m.functions[0].allocations:
        if not isinstance(alloc, mybir.MemoryLocationSet):
            continue
        name = alloc.memorylocations[0].name
        if alloc.kind == "ExternalInput":
            if name != partition_name:
                in_names.append(name)
        elif alloc.kind == "ExternalOutput":
            out_names.append(name)
            out_avals.append(
                jax.core.ShapedArray(
                    tuple(alloc.tensor_shape), mybir.dt.np(alloc.dtype)
                )
            )

    sharded_inputs = {"xT"}
    call_names = tuple(in_names) + tuple(out_names)
    if partition_name is not None:
        call_names = call_names + (partition_name,)

    def _body(*args):
        extra = [partition_id_tensor()] if partition_name is not None else []
        outs = _bass_exec_p.bind(
            *args,
            *extra,
            out_avals=tuple(out_avals),
            in_names=call_names,
            out_names=tuple(out_names),
            lowering_input_output_aliases=(),
            sim_require_finite=True,
            sim_require_nnan=True,
            nc=nc,
        )
        return tuple(outs)

    devices = jax.devices()[:N_CORES]
    mesh = Mesh(np.asarray(devices), ("core",))
    in_specs = tuple(
        PartitionSpec("core") if n in sharded_inputs else PartitionSpec()
        for n in in_names
    ) + (PartitionSpec("core"),) * len(out_names)
    out_specs = (PartitionSpec("core"),) * len(out_names)
    fn = jax.jit(
        shard_map(
            _body, mesh=mesh, in_specs=in_specs, out_specs=out_specs, check_rep=False
        )
    )

    # device-resident zero output buffers, transferred once and reused
    zeros = [
        jax.device_put(
            np.zeros((N_CORES * a.shape[0], *a.shape[1:]), a.dtype),
            NamedSharding(mesh, PartitionSpec("core")),
        )
        for a in out_avals
    ]
    return fn, in_names, zeros


def _get_runner():
    if "runner" not in _cached:
        _cached["runner"] = _make_runner()
    return _cached["runner"]


def _host_prep(x, projector, d, b):
    """Build device inputs; the SORF fold itself happens on device."""
    H = np.ascontiguousarray(_hadamard128(), dtype=np.float32)
    d32 = d.astype(np.float32)
    d0 = np.ascontiguousarray(d32[0].T * np.float32(C_SCALE))  # [128, 64], scaled
    d1 = np.ascontiguousarray(d32[1].T)  # [128, 64]

    # phase bias in periods: b' = b + 0.25 (cos -> sin); b'' = b' - round(b')
    bp = b.astype(np.float64) + 0.25
    bpp = bp - np.round(bp)  # in [-0.5, 0.5]
    b1 = np.ascontiguousarray(bpp.reshape(NSTACKS, 128).T.astype(np.float32))
    b2 = np.ascontiguousarray(
        (TWO_PI * bpp).reshape(NSTACKS, 128).T.astype(np.float32)
    )

    Pc = np.ascontiguousarray(projector.astype(np.float32).reshape(4, 128, 128))

    # global xT: [8*4, 128, 1024]; shard_map slices axis 0 per core
    x2 = x.astype(np.float32).reshape(ROWS, 512)
    xT = np.empty((N_CORES, 4, 128, ROWS_PER_CORE), np.float32)
    for core in range(N_CORES):
        xs = x2[core * ROWS_PER_CORE : (core + 1) * ROWS_PER_CORE]
        xT[core] = xs.T.reshape(4, 128, ROWS_PER_CORE)
    xT = xT.reshape(N_CORES * 4, 128, ROWS_PER_CORE)
    return {
        "xT": xT, "Pc": Pc, "Hd": H, "d0d": d0, "d1d": d1, "b1d": b1, "b2d": b2
    }


def _assemble(out_global):
    """core-sharded [8*64, 2, 128, 512] bf16 -> [64, 128, 8192] fp32."""
    full = np.empty((ROWS, OUT_DIM), np.float32)
    view = full.reshape(N_CORES, N_CHUNKS, CHUNK, NSTACKS, 128)

    shards = sorted(
        out_global.addressable_shards, key=lambda s: s.index[0].start or 0
    )

    def fetch(i):
        o = np.asarray(shards[i].data)  # [64, 2, 128, 512] bf16
        # o[s, c, m, j] -> view[i, c, j, s, m], upcast bf16 -> fp32
        np.copyto(view[i], o.transpose(1, 3, 0, 2))

    with _futures.ThreadPoolExecutor(max_workers=N_CORES) as ex:
        list(ex.map(fetch, range(N_CORES)))
    return full.reshape(64, 128, OUT_DIM)


def kernel(x, projector, d, b):
    fn, in_names, zeros = _get_runner()
    ins = _host_prep(
        np.asarray(x), np.asarray(projector), np.asarray(d), np.asarray(b)
    )
    outs = fn(*[ins[n] for n in in_names], *zeros)
    return _assemble(outs[0])
